# revision 1
# baseline (speedup 1.0000x reference)
"""Trainium2 Bass kernel for nn_Conv2d_61881888800824.

StyleGAN2-style synthesis layer:
    renorm(weight) -> upsample x2 (zero-insert) -> 4x4 FIR -> 3x3 conv
    -> + noise*strength -> + bias -> leaky_relu(0.2) * sqrt(2)

Math: the upsample+FIR+conv chain is folded (host-side) into a polyphase
decomposition — 4 independent 3x3 convolutions of the ORIGINAL 64x64 input
(one per output-pixel parity phase (alpha,beta)), each expressed as
channel-contraction matmuls on the TensorEngine in float32r (tf32-like)
precision with fp32 PSUM accumulation. The sqrt(2) lrelu gain is folded
into weights/bias/noise (lrelu is positively homogeneous).

Sharding: data-parallel over batch — 1 image per NeuronCore, 8 cores.

Self-contained: inputs are the full tensors from setup_inputs(); output is
the full [8, 256, 128, 128] fp32 array.
"""
from contextlib import ExitStack

import numpy as np

import bass_rust
import concourse.bass as bass
import concourse.mybir as mybir
import concourse.tile as tile
from concourse.bass_utils import run_bass_kernel_spmd

F32R = mybir.dt.float32r
F32 = mybir.dt.float32


# ---------------------------------------------------------------------------
# Wait legalization: this walrus build accepts at most ONE embedded sync wait
# per instruction. Tile can emit more (incl. same-engine self-waits that are
# provably satisfied by the engine's serial program order). Drop the provable
# ones; move the rest onto standalone EventSemaphore instructions inserted
# just before the over-limit instruction on the same engine.
# ---------------------------------------------------------------------------

def _is_async_update(inst) -> bool:
    n = type(inst).__name__
    return 'DMA' in n or 'Swdge' in n or 'Collective' in n or 'Dma' in n


def legalize_waits(nc, evsem_limit: int = 1) -> int:
    n_fixed = 0
    for fn in nc.m.functions:
        for bb in fn.blocks:
            insts = bb.instructions
            cum: dict[tuple, int] = {}
            out = []
            changed = False
            for inst in insts:
                si = inst.sync_info
                waits = list(si.on_wait) if si is not None and si.on_wait else []
                updates = list(si.on_update) if si is not None and si.on_update else []
                eng = inst.engine
                limit = 1
                if len(waits) > limit:
                    kept = []
                    for w in waits:
                        if (w.sync_type == 'semaphore'
                                and w.wait_mode == 'sem-ge-imm'
                                and w.wait_reg is None
                                and cum.get((eng, w.id), 0) >= w.wait_value):
                            continue
                        kept.append(w)
                    waits = kept
                if len(waits) > limit:
                    excess = waits[:-limit]
                    waits = waits[-limit:]
                    while excess:
                        take, excess = excess[:evsem_limit], excess[evsem_limit:]
                        ev = mybir.InstEventSemaphore(
                            name=nc.get_next_instruction_name(), ins=[], outs=[])
                        ev.engine = eng
                        ev.sync_info = bass_rust.SyncInfo(on_wait=take, on_update=[])
                        out.append(ev)
                    inst.sync_info = bass_rust.SyncInfo(on_wait=waits,
                                                        on_update=updates)
                    changed = True
                    n_fixed += 1
                elif si is not None and len(list(si.on_wait or [])) != len(waits):
                    inst.sync_info = bass_rust.SyncInfo(on_wait=waits,
                                                        on_update=updates)
                    changed = True
                    n_fixed += 1
                out.append(inst)
                if not _is_async_update(inst):
                    for u in updates:
                        if (u.sync_type == 'semaphore'
                                and u.update_mode == 'sem-inc'
                                and u.update_reg is None):
                            k = (eng, u.id)
                            cum[k] = cum.get(k, 0) + u.update_value
            if changed:
                bb.instructions = out
    return n_fixed


# ---------------------------------------------------------------------------
# Device kernel (per core: one batch image)
# ---------------------------------------------------------------------------

def build_conv_nc(mm_dtype=F32R):
    nc = bass.Bass("TRN2", target_bir_lowering=False, debug=False)
    xin = nc.dram_tensor("xin", [512, 64, 64], mm_dtype, kind="ExternalInput").ap()
    wq = nc.dram_tensor("wq", [2, 2, 128, 72 * 128], mm_dtype,
                        kind="ExternalInput").ap()
    noise4 = nc.dram_tensor("noise4", [2, 2, 64, 64], F32, kind="ExternalInput").ap()
    biasq = nc.dram_tensor("biasq", [128, 2], F32, kind="ExternalInput").ap()
    y = nc.dram_tensor("y", [256, 128, 128], F32, kind="ExternalOutput").ap()

    with ExitStack() as ctx:
        tc = ctx.enter_context(tile.TileContext(nc))
        xp = ctx.enter_context(tc.tile_pool(name="x", bufs=1))
        wp = ctx.enter_context(tc.tile_pool(name="w", bufs=2))
        np_ = ctx.enter_context(tc.tile_pool(name="noise", bufs=1))
        bp = ctx.enter_context(tc.tile_pool(name="bias", bufs=1))
        op = ctx.enter_context(tc.tile_pool(name="out", bufs=3))
        pp = ctx.enter_context(tc.tile_pool(name="psum", bufs=8, space="PSUM"))

        bias_sb = bp.tile([128, 2], F32)
        nc.sync.dma_start(bias_sb[:], biasq)

        # hoist the first weight slab ahead of the x loads so the SDMA
        # round-robin finishes the first-matmul critical path sooner
        wslab0 = wp.tile([128, 72, 128], mm_dtype, tag="wslab")
        wsrc0 = wq[0, 0].rearrange("ci (j co) -> ci j co", co=128)
        nc.sync.dma_start(wslab0[:, 0:36, :], wsrc0[:, 0:36, :])
        nc.sync.dma_start(wslab0[:, 36:72, :], wsrc0[:, 36:72, :])

        # x with 1-pixel zero border: [128, cg, 66, 66]; row-half DMAs give
        # finer dependency granularity for the first accumulation groups
        xq = xp.tile([128, 4, 66, 66], mm_dtype)
        for cg in range(4):
            for sl in (xq[:, cg, 0, :], xq[:, cg, 65, :],
                       xq[:, cg, :, 0], xq[:, cg, :, 65]):
                nc.vector.memset(sl.bitcast(F32), 0.0)
        for cg in range(4):
            nc.sync.dma_start(xq[:, cg, 1:33, 1:65],
                              xin[cg * 128:(cg + 1) * 128, 0:32])
            nc.sync.dma_start(xq[:, cg, 33:65, 1:65],
                              xin[cg * 128:(cg + 1) * 128, 32:64])

        for a in range(2):          # output row parity (alpha)
            noise_sb = np_.tile([128, 2, 64, 64], F32)
            nsrc = bass.AP(
                tensor=noise4.tensor,
                offset=a * 2 * 4096,
                ap=[[0, 128], [4096, 2], [64, 64], [1, 64]],
            )
            nc.sync.dma_start(noise_sb[:], nsrc)
            for ct in range(2):     # cout tile of 128
                if a == 0 and ct == 0:
                    wslab = wslab0
                else:
                    wslab = wp.tile([128, 72, 128], mm_dtype, tag="wslab")
                    wsrc = wq[a, ct].rearrange("ci (j co) -> ci j co", co=128)
                    # per beta-half: first matmuls only wait for half
                    nc.sync.dma_start(wslab[:, 0:36, :], wsrc[:, 0:36, :])
                    nc.sync.dma_start(wslab[:, 36:72, :], wsrc[:, 36:72, :])
                for c in range(8):  # chunk of 8 output-phase rows
                    ot = op.tile([128, 8, 128], F32)
                    for b in range(2):   # output col parity (beta)
                        ps = pp.tile([128, 8, 64], F32)
                        for cg in range(4):
                            for tap in range(9):
                                ky, kx = tap // 3, tap % 3
                                j = b * 36 + cg * 9 + tap
                                nc.tensor.matmul(
                                    ps[:],
                                    wslab[:, j, :],
                                    xq[:, cg, 8 * c + ky:8 * c + ky + 8,
                                       kx:kx + 64],
                                    start=(cg == 0 and tap == 0),
                                    stop=(cg == 3 and tap == 8),
                                )
                        # psum += noise; z = psum + bias -> ot (strided by beta)
                        nc.vector.tensor_add(
                            ps[:], ps[:], noise_sb[:, b, 8 * c:8 * c + 8, :])
                        nc.scalar.activation(
                            ot[:, :, b::2], ps[:],
                            mybir.ActivationFunctionType.Identity,
                            bias=bias_sb[:, ct:ct + 1], scale=1.0)
                    # leaky relu: ot = max(0.2*ot, ot)
                    nc.vector.scalar_tensor_tensor(
                        ot[:], ot[:], 0.2, ot[:],
                        mybir.AluOpType.mult, mybir.AluOpType.max)
                    ydst = bass.AP(
                        tensor=y.tensor,
                        offset=(ct * 128) * 16384 + (16 * c + a) * 128,
                        ap=[[16384, 128], [256, 8], [1, 128]],
                    )
                    nc.sync.dma_start(ydst, ot[:])
    legalize_waits(nc)
    return nc


# ---------------------------------------------------------------------------
# Host-side preparation (weight renorm + FIR folding + phase decomposition)
# ---------------------------------------------------------------------------

def prep_inputs(x, weight, bias, noise_const, noise_strength):
    SQ2 = np.sqrt(2.0)
    w = np.asarray(weight).astype(np.float64)
    inv = 1.0 / np.sqrt((w ** 2).sum(axis=(1, 2, 3)) + 1e-8)
    w = w * inv[:, None, None, None]
    f = np.array([1., 3., 3., 1.])
    f = np.outer(f, f)
    f = f / f.sum() * 4.0                       # FIR * up^2 gain
    wf = w[:, :, ::-1, ::-1]                    # flipped (cross-corr of flip)
    g = np.zeros((w.shape[0], w.shape[1], 6, 6))
    for m in range(3):
        for n in range(3):
            g[:, :, m:m + 4, n:n + 4] += wf[:, :, m, n, None, None] * f
    # wq[a, ct, ci, b*36 + cg*9 + tap, co]
    wq = np.empty((2, 2, 128, 72, 128), dtype=np.float32)
    for a in range(2):
        for b in range(2):
            h = g[:, :, (1 - a)::2, (1 - b)::2] * SQ2    # [Cout, Cin, 3, 3]
            h5 = h.reshape(2, 128, 4, 128, 9)            # [ct, co, cg, ci, tap]
            h5 = h5.transpose(0, 3, 2, 4, 1)             # [ct, ci, cg, tap, co]
            wq[a, :, :, b * 36:(b + 1) * 36, :] = h5.reshape(2, 128, 36, 128)
    wq = np.ascontiguousarray(wq.reshape(2, 2, 128, 72 * 128), dtype=np.float32)

    noise2 = np.asarray(noise_const).astype(np.float64) * float(noise_strength) * SQ2
    noise4 = np.empty((2, 2, 64, 64), dtype=np.float32)
    for a in range(2):
        for b in range(2):
            noise4[a, b] = noise2[a::2, b::2]

    biasq = np.empty((128, 2), dtype=np.float32)
    bias2 = np.asarray(bias).astype(np.float64) * SQ2
    biasq[:, 0] = bias2[:128]
    biasq[:, 1] = bias2[128:]

    x = np.asarray(x)
    return [{
        "xin": np.ascontiguousarray(x[bi], dtype=np.float32),
        "wq": wq,
        "noise4": noise4,
        "biasq": biasq,
    } for bi in range(x.shape[0])]


_NC_CACHE = None


def kernel(x, weight, bias, noise_const, noise_strength):
    global _NC_CACHE
    if _NC_CACHE is None:
        _NC_CACHE = build_conv_nc()
    in_maps = prep_inputs(x, weight, bias, noise_const, noise_strength)
    res = run_bass_kernel_spmd(_NC_CACHE, in_maps, core_ids=list(range(8)))
    return np.ascontiguousarray(
        np.stack([r["y"] for r in res.results]), dtype=np.float32)



# revision 2
# speedup vs baseline: 2.1680x; 2.1680x over previous
"""Trainium2 Bass kernel for nn_Conv2d_61881888800824.

StyleGAN2-style synthesis layer:
    renorm(weight) -> upsample x2 (zero-insert) -> 4x4 FIR -> 3x3 conv
    -> + noise*strength -> + bias -> leaky_relu(0.2) * sqrt(2)

Math: convolution associativity lets the 3x3 conv run BEFORE the FIR:
    y = (pad3(U2(x)) corr wf) corr f
so the TensorEngine only contracts the original 9 taps (polyphase over the
4 output parities of z = pad3(U2 x) corr wf: tap counts 1/2/2/4), a 4x
MAC reduction vs folding the FIR into the conv weights. The separable
4x4 FIR [1,3,3,1]^2 then runs on the Vector engine in fp16 (2 adds + a
x3 prescale per dimension), the per-pixel noise add on GpSimd, and the
scale + bias + leaky-relu + fp32 conversion in a single ScalarEngine
Prelu activation.

Sharding: data-parallel over batch — 1 image per NeuronCore, 8 cores.

Self-contained: inputs are the full tensors from setup_inputs(); output is
the full [8, 256, 128, 128] fp32 array.
"""
from contextlib import ExitStack

import numpy as np

import bass_rust
import concourse.bass as bass
import concourse.mybir as mybir
import concourse.tile as tile
from concourse.bass_utils import run_bass_kernel_spmd

F16 = mybir.dt.float16
F32 = mybir.dt.float32

ADD = mybir.AluOpType.add


# ---------------------------------------------------------------------------
# Wait legalization: this walrus build accepts at most ONE embedded sync wait
# per instruction. Tile can emit more (incl. same-engine self-waits that are
# provably satisfied by the engine's serial program order). Drop the provable
# ones; move the rest onto standalone EventSemaphore instructions inserted
# just before the over-limit instruction on the same engine.
# ---------------------------------------------------------------------------

def _is_async_update(inst) -> bool:
    n = type(inst).__name__
    return 'DMA' in n or 'Swdge' in n or 'Collective' in n or 'Dma' in n


def legalize_waits(nc, evsem_limit: int = 1) -> int:
    n_fixed = 0
    for fn in nc.m.functions:
        for bb in fn.blocks:
            insts = bb.instructions
            cum: dict[tuple, int] = {}
            out = []
            changed = False
            for inst in insts:
                si = inst.sync_info
                waits = list(si.on_wait) if si is not None and si.on_wait else []
                updates = list(si.on_update) if si is not None and si.on_update else []
                eng = inst.engine
                limit = 1
                if len(waits) > limit:
                    kept = []
                    for w in waits:
                        if (w.sync_type == 'semaphore'
                                and w.wait_mode == 'sem-ge-imm'
                                and w.wait_reg is None
                                and cum.get((eng, w.id), 0) >= w.wait_value):
                            continue
                        kept.append(w)
                    waits = kept
                if len(waits) > limit:
                    excess = waits[:-limit]
                    waits = waits[-limit:]
                    while excess:
                        take, excess = excess[:evsem_limit], excess[evsem_limit:]
                        ev = mybir.InstEventSemaphore(
                            name=nc.get_next_instruction_name(), ins=[], outs=[])
                        ev.engine = eng
                        ev.sync_info = bass_rust.SyncInfo(on_wait=take, on_update=[])
                        out.append(ev)
                    inst.sync_info = bass_rust.SyncInfo(on_wait=waits,
                                                        on_update=updates)
                    changed = True
                    n_fixed += 1
                elif si is not None and len(list(si.on_wait or [])) != len(waits):
                    inst.sync_info = bass_rust.SyncInfo(on_wait=waits,
                                                        on_update=updates)
                    changed = True
                    n_fixed += 1
                out.append(inst)
                if not _is_async_update(inst):
                    for u in updates:
                        if (u.sync_type == 'semaphore'
                                and u.update_mode == 'sem-inc'
                                and u.update_reg is None):
                            k = (eng, u.id)
                            cum[k] = cum.get(k, 0) + u.update_value
            if changed:
                bb.instructions = out
    return n_fixed


# ---------------------------------------------------------------------------
# Device kernel (per core: one batch image)
# ---------------------------------------------------------------------------

# per-dim polyphase taps of the 3x3 kernel on the zero-stuffed fine grid:
# output parity 0 sees tap k=1 at xpad offset 0; parity 1 sees taps k=0
# (offset 0) and k=2 (offset 1).  xpad[r] = x[r-1] with a zero border.
ROWTAPS = {0: [(1, 0)], 1: [(0, 0), (2, 1)]}

ACT_SCALE = float(np.sqrt(2.0) / 16.0)   # sqrt2 lrelu gain / FIR x16 raw scale

# FIR chunk emission: after matmul row-block k (fine rows 12k..12k+11),
# which row-pass / col-pass chunks (16 output rows each) are ready.
FIR_SCHED = {
    1: [('row', 0)], 2: [('row', 1), ('col', 0)], 3: [('row', 2), ('col', 1)],
    5: [('row', 3), ('col', 2)], 6: [('row', 4), ('col', 3)],
    7: [('row', 5), ('col', 4)], 9: [('row', 6), ('col', 5)],
    10: [('row', 7), ('col', 6), ('col', 7)],
}


def build_conv_nc():
    nc = bass.Bass("TRN2", target_bir_lowering=False, debug=False)
    xin = nc.dram_tensor("xin", [512, 64, 64], F16, kind="ExternalInput").ap()
    wq = nc.dram_tensor("wq", [128, 2, 4, 9, 128], F16,
                        kind="ExternalInput").ap()
    noise = nc.dram_tensor("noise", [128, 128], F16, kind="ExternalInput").ap()
    biasq = nc.dram_tensor("biasq", [128, 2], F32, kind="ExternalInput").ap()
    y = nc.dram_tensor("y", [256, 128, 128], F32, kind="ExternalOutput").ap()

    with ExitStack() as ctx:
        tc = ctx.enter_context(tile.TileContext(nc))
        xp = ctx.enter_context(tc.tile_pool(name="x", bufs=1))
        wp = ctx.enter_context(tc.tile_pool(name="w", bufs=1))
        bp = ctx.enter_context(tc.tile_pool(name="bias", bufs=1))
        zp = ctx.enter_context(tc.tile_pool(name="z", bufs=1))
        tp = ctx.enter_context(tc.tile_pool(name="t", bufs=1))
        slp = ctx.enter_context(tc.tile_pool(name="sliver", bufs=2))
        adp = ctx.enter_context(tc.tile_pool(name="adds", bufs=4))
        yrp = ctx.enter_context(tc.tile_pool(name="yraw", bufs=2))
        nzp = ctx.enter_context(tc.tile_pool(name="nz", bufs=2))
        nap = ctx.enter_context(tc.tile_pool(name="nadd", bufs=2))
        ysp = ctx.enter_context(tc.tile_pool(name="ystage", bufs=2))
        pp = ctx.enter_context(tc.tile_pool(name="psum", bufs=8, space="PSUM"))

        bias_sb = bp.tile([128, 2], F32)
        nc.sync.dma_start(bias_sb[:], biasq)
        wsb = wp.tile([128, 2, 4, 9, 128], F16)
        nc.sync.dma_start(wsb[:, 0], wq[:, 0])
        nc.sync.dma_start(wsb[:, 1], wq[:, 1])

        # x with zero border: xpad[:, cg, r, c] = x[cg*128.., r-1, c-1],
        # rows/cols 1..64 interior, zeros at 0 and 65/66 (col 67 unused pad).
        xq = xp.tile([128, 4, 67, 68], F16)
        for cg in range(4):
            nc.vector.memset(xq[:, cg, 0, 0:68], 0.0)
            nc.vector.memset(xq[:, cg, 65:67, 0:68], 0.0)
            nc.vector.memset(xq[:, cg, 1:65, 0], 0.0)
            nc.vector.memset(xq[:, cg, 1:65, 65:68], 0.0)
        for cg in range(4):
            nc.sync.dma_start(xq[:, cg, 1:33, 1:65],
                              xin[cg * 128:(cg + 1) * 128, 0:32])
            nc.sync.dma_start(xq[:, cg, 33:65, 1:65],
                              xin[cg * 128:(cg + 1) * 128, 32:64])

        zt = zp.tile([128, 132, 132], F16)
        tt = tp.tile([128, 131, 128], F16)

        def emit_block(ct, k):
            """Matmuls + psum->z copies for fine rows 12k..12k+11."""
            for a in range(2):
                for b in range(2):
                    ps = pp.tile([128, 6, 66], F32)
                    taps = [(ky, ry, kx, rx)
                            for (ky, ry) in ROWTAPS[a]
                            for (kx, rx) in ROWTAPS[b]]
                    n = len(taps) * 4
                    i = 0
                    for cg in range(4):
                        for (ky, ry, kx, rx) in taps:
                            nc.tensor.matmul(
                                ps[:],
                                wsb[:, ct, cg, ky * 3 + kx, :],
                                xq[:, cg, 6 * k + ry:6 * k + ry + 6,
                                   rx:rx + 66],
                                start=(i == 0), stop=(i == n - 1))
                            i += 1
                    nc.scalar.copy(
                        zt[:, 12 * k + a:12 * k + 12:2, b:132:2], ps[:])

        def emit_row_chunk(i):
            """Horizontal FIR t[r,c] = (z[c]+3z[c+1]) + (3z[c+2]+z[c+3])."""
            r0, nr = 16 * i, (16 if i < 7 else 19)
            z3 = slp.tile([128, 19, 132], F16, tag="sl")
            nc.vector.tensor_scalar_mul(z3[:, 0:nr, :], zt[:, r0:r0 + nr, :],
                                        3.0)
            u = adp.tile([128, 19, 128], F16, tag="ad")
            nc.vector.tensor_tensor(u[:, 0:nr, :], zt[:, r0:r0 + nr, 0:128],
                                    z3[:, 0:nr, 1:129], ADD)
            v = adp.tile([128, 19, 128], F16, tag="ad")
            nc.vector.tensor_tensor(v[:, 0:nr, :], z3[:, 0:nr, 2:130],
                                    zt[:, r0:r0 + nr, 3:131], ADD)
            nc.vector.tensor_tensor(tt[:, r0:r0 + nr, :], u[:, 0:nr, :],
                                    v[:, 0:nr, :], ADD)

        def emit_col_chunk(ct, i):
            """Vertical FIR + noise + prelu epilogue for out rows 16i..+16."""
            r0 = 16 * i
            t3 = slp.tile([128, 19, 128], F16, tag="sl")
            nc.vector.tensor_scalar_mul(t3[:], tt[:, r0:r0 + 19, :], 3.0)
            u2 = adp.tile([128, 19, 128], F16, tag="ad")
            nc.vector.tensor_tensor(u2[:, 0:16, :], tt[:, r0:r0 + 16, :],
                                    t3[:, 1:17, :], ADD)
            v2 = adp.tile([128, 19, 128], F16, tag="ad")
            nc.vector.tensor_tensor(v2[:, 0:16, :], t3[:, 2:18, :],
                                    tt[:, r0 + 3:r0 + 19, :], ADD)
            yr = yrp.tile([128, 16, 128], F16)
            nc.vector.tensor_tensor(yr[:], u2[:, 0:16, :], v2[:, 0:16, :], ADD)
            nzt = nzp.tile([128, 16, 128], F16)
            nsrc = bass.AP(tensor=noise.tensor, offset=r0 * 128,
                           ap=[[0, 128], [128, 16], [1, 128]])
            nc.sync.dma_start(nzt[:], nsrc)
            na = nap.tile([128, 16, 128], F16)
            nc.gpsimd.tensor_tensor(na[:], yr[:], nzt[:], ADD)
            ys = ysp.tile([128, 16, 128], F32)
            nc.scalar.activation(ys[:], na[:],
                                 mybir.ActivationFunctionType.Prelu,
                                 bias=bias_sb[:, ct:ct + 1], scale=ACT_SCALE,
                                 alpha=0.2)
            ydst = bass.AP(tensor=y.tensor,
                           offset=(ct * 128) * 16384 + r0 * 128,
                           ap=[[16384, 128], [128, 16], [1, 128]])
            nc.sync.dma_start(ydst, ys[:])

        for ct in range(2):
            for k in range(11):
                emit_block(ct, k)
                for kind, i in FIR_SCHED.get(k, []):
                    if kind == 'row':
                        emit_row_chunk(i)
                    else:
                        emit_col_chunk(ct, i)
    legalize_waits(nc)
    return nc


# ---------------------------------------------------------------------------
# Host-side preparation (weight renorm + polyphase layout, fp16 conversion)
# ---------------------------------------------------------------------------

def prep_inputs(x, weight, bias, noise_const, noise_strength):
    SQ2 = np.sqrt(2.0)
    w = np.asarray(weight).astype(np.float64)
    inv = 1.0 / np.sqrt((w ** 2).sum(axis=(1, 2, 3)) + 1e-8)
    w = w * inv[:, None, None, None]
    wf = w[:, :, ::-1, ::-1]                    # flipped (cross-corr of flip)

    # wq[ci_local, ct, cg, ky*3+kx, co] = wf[ct*128+co, cg*128+ci, ky, kx]
    wq = np.empty((128, 2, 4, 9, 128), dtype=np.float16)
    for ct in range(2):
        for cg in range(4):
            blk = wf[ct * 128:(ct + 1) * 128, cg * 128:(cg + 1) * 128]
            wq[:, ct, cg, :, :] = blk.transpose(1, 2, 3, 0).reshape(128, 9, 128)

    # noise pre-scaled so the Act scale sqrt2/16 lands it at
    # sqrt2 * strength * noise:  noise_pre = 16 * strength * noise
    noise_pre = (np.asarray(noise_const).astype(np.float64)
                 * float(noise_strength) * 16.0).astype(np.float16)

    biasq = np.empty((128, 2), dtype=np.float32)
    bias2 = np.asarray(bias).astype(np.float64) * SQ2
    biasq[:, 0] = bias2[:128]
    biasq[:, 1] = bias2[128:]

    x = np.asarray(x)
    return [{
        "xin": x[bi].astype(np.float16),
        "wq": wq,
        "noise": noise_pre,
        "biasq": biasq,
    } for bi in range(x.shape[0])]


_NC_CACHE = None


def kernel(x, weight, bias, noise_const, noise_strength):
    global _NC_CACHE
    if _NC_CACHE is None:
        _NC_CACHE = build_conv_nc()
    in_maps = prep_inputs(x, weight, bias, noise_const, noise_strength)
    res = run_bass_kernel_spmd(_NC_CACHE, in_maps, core_ids=list(range(8)))
    return np.ascontiguousarray(
        np.stack([r["y"] for r in res.results]), dtype=np.float32)


# revision 5
# speedup vs baseline: 2.1699x; 1.0009x over previous
"""Trainium2 Bass kernel for nn_Conv2d_61881888800824.

StyleGAN2-style synthesis layer:
    renorm(weight) -> upsample x2 (zero-insert) -> 4x4 FIR -> 3x3 conv
    -> + noise*strength -> + bias -> leaky_relu(0.2) * sqrt(2)

Math: convolution associativity lets the 3x3 conv run BEFORE the FIR:
    y = (pad3(U2(x)) corr wf) corr f
so the TensorEngine only contracts the original 9 taps (polyphase over the
4 output parities of z = pad3(U2 x) corr wf: tap counts 1/2/2/4), a 4x
MAC reduction vs folding the FIR into the conv weights. The separable
4x4 FIR [1,3,3,1]^2 then runs on the Vector engine in fp16 (2 adds + a
x3 prescale per dimension), the per-pixel noise add on GpSimd, and the
scale + bias + leaky-relu + fp32 conversion in a single ScalarEngine
Prelu activation.

Sharding: data-parallel over batch — 1 image per NeuronCore, 8 cores.

Self-contained: inputs are the full tensors from setup_inputs(); output is
the full [8, 256, 128, 128] fp32 array.
"""
from contextlib import ExitStack

import numpy as np

import bass_rust
import concourse.bass as bass
import concourse.mybir as mybir
import concourse.tile as tile
from concourse.bass_utils import run_bass_kernel_spmd

F16 = mybir.dt.float16
F32 = mybir.dt.float32

ADD = mybir.AluOpType.add


# ---------------------------------------------------------------------------
# Wait legalization: this walrus build accepts at most ONE embedded sync wait
# per instruction. Tile can emit more (incl. same-engine self-waits that are
# provably satisfied by the engine's serial program order). Drop the provable
# ones; move the rest onto standalone EventSemaphore instructions inserted
# just before the over-limit instruction on the same engine.
# ---------------------------------------------------------------------------

def _is_async_update(inst) -> bool:
    n = type(inst).__name__
    return 'DMA' in n or 'Swdge' in n or 'Collective' in n or 'Dma' in n


def legalize_waits(nc, evsem_limit: int = 1) -> int:
    n_fixed = 0
    for fn in nc.m.functions:
        for bb in fn.blocks:
            insts = bb.instructions
            cum: dict[tuple, int] = {}
            out = []
            changed = False
            for inst in insts:
                si = inst.sync_info
                waits = list(si.on_wait) if si is not None and si.on_wait else []
                updates = list(si.on_update) if si is not None and si.on_update else []
                eng = inst.engine
                limit = 1
                if len(waits) > limit:
                    kept = []
                    for w in waits:
                        if (w.sync_type == 'semaphore'
                                and w.wait_mode == 'sem-ge-imm'
                                and w.wait_reg is None
                                and cum.get((eng, w.id), 0) >= w.wait_value):
                            continue
                        kept.append(w)
                    waits = kept
                if len(waits) > limit:
                    excess = waits[:-limit]
                    waits = waits[-limit:]
                    while excess:
                        take, excess = excess[:evsem_limit], excess[evsem_limit:]
                        ev = mybir.InstEventSemaphore(
                            name=nc.get_next_instruction_name(), ins=[], outs=[])
                        ev.engine = eng
                        ev.sync_info = bass_rust.SyncInfo(on_wait=take, on_update=[])
                        out.append(ev)
                    inst.sync_info = bass_rust.SyncInfo(on_wait=waits,
                                                        on_update=updates)
                    changed = True
                    n_fixed += 1
                elif si is not None and len(list(si.on_wait or [])) != len(waits):
                    inst.sync_info = bass_rust.SyncInfo(on_wait=waits,
                                                        on_update=updates)
                    changed = True
                    n_fixed += 1
                out.append(inst)
                if not _is_async_update(inst):
                    for u in updates:
                        if (u.sync_type == 'semaphore'
                                and u.update_mode == 'sem-inc'
                                and u.update_reg is None):
                            k = (eng, u.id)
                            cum[k] = cum.get(k, 0) + u.update_value
            if changed:
                bb.instructions = out
    return n_fixed


# ---------------------------------------------------------------------------
# Device kernel (per core: one batch image)
# ---------------------------------------------------------------------------

# per-dim polyphase taps of the 3x3 kernel on the zero-stuffed fine grid:
# output parity 0 sees tap k=1 at xpad offset 0; parity 1 sees taps k=0
# (offset 0) and k=2 (offset 1).  xpad[r] = x[r-1] with a zero border.
ROWTAPS = {0: [(1, 0)], 1: [(0, 0), (2, 1)]}

ACT_SCALE = float(np.sqrt(2.0) / 16.0)   # sqrt2 lrelu gain / FIR x16 raw scale

# FIR chunk emission: after matmul row-block k (fine rows 12k..12k+11),
# which row-pass / col-pass chunks (16 output rows each) have their inputs.
ROW_SCHED = {1: [0], 2: [1], 3: [2], 5: [3], 6: [4], 7: [5], 9: [6], 10: [7]}
COL_SCHED = {2: [0], 3: [1], 5: [2], 6: [3], 7: [4], 9: [5], 10: [6, 7]}
# The Prelu + output-DMA half of a col chunk is emitted EPI_DELAY block
# slots later so the scalar engine's in-order queue never parks a Prelu
# (late dependency) ahead of psum->z copies the TensorEngine is waiting on.
EPI_DELAY = 2


def build_conv_nc():
    nc = bass.Bass("TRN2", target_bir_lowering=False, debug=False)
    xin = nc.dram_tensor("xin", [512, 64, 64], F16, kind="ExternalInput").ap()
    wq = nc.dram_tensor("wq", [128, 2, 4, 9, 128], F16,
                        kind="ExternalInput").ap()
    noise = nc.dram_tensor("noise", [128, 128], F16, kind="ExternalInput").ap()
    biasq = nc.dram_tensor("biasq", [128, 2], F32, kind="ExternalInput").ap()
    y = nc.dram_tensor("y", [256, 128, 128], F32, kind="ExternalOutput").ap()

    with ExitStack() as ctx:
        tc = ctx.enter_context(tile.TileContext(nc))
        xp = ctx.enter_context(tc.tile_pool(name="x", bufs=1))
        wp = ctx.enter_context(tc.tile_pool(name="w", bufs=1))
        bp = ctx.enter_context(tc.tile_pool(name="bias", bufs=1))
        zp = ctx.enter_context(tc.tile_pool(name="z", bufs=1))
        tp = ctx.enter_context(tc.tile_pool(name="t", bufs=1))
        slp = ctx.enter_context(tc.tile_pool(name="sliver", bufs=2))
        adp = ctx.enter_context(tc.tile_pool(name="adds", bufs=4))
        yrp = ctx.enter_context(tc.tile_pool(name="yraw", bufs=2))
        nzp = ctx.enter_context(tc.tile_pool(name="nz", bufs=2))
        nap = ctx.enter_context(tc.tile_pool(name="nadd", bufs=3))
        ysp = ctx.enter_context(tc.tile_pool(name="ystage", bufs=2))
        pp = ctx.enter_context(tc.tile_pool(name="psum", bufs=8, space="PSUM"))

        bias_sb = bp.tile([128, 2], F32)
        nc.sync.dma_start(bias_sb[:], biasq)
        wsb = wp.tile([128, 2, 4, 9, 128], F16)
        nc.sync.dma_start(wsb[:, 0], wq[:, 0])
        nc.sync.dma_start(wsb[:, 1], wq[:, 1])

        # x with zero border: xpad[:, cg, r, c] = x[cg*128.., r-1, c-1],
        # rows/cols 1..64 interior, zeros at 0 and 65/66 (col 67 unused pad).
        xq = xp.tile([128, 4, 67, 68], F16)
        for cg in range(4):
            nc.vector.memset(xq[:, cg, 0, 0:68], 0.0)
            nc.vector.memset(xq[:, cg, 65:67, 0:68], 0.0)
            nc.vector.memset(xq[:, cg, 1:65, 0], 0.0)
            nc.vector.memset(xq[:, cg, 1:65, 65:68], 0.0)
        for cg in range(4):
            nc.sync.dma_start(xq[:, cg, 1:33, 1:65],
                              xin[cg * 128:(cg + 1) * 128, 0:32])
            nc.sync.dma_start(xq[:, cg, 33:65, 1:65],
                              xin[cg * 128:(cg + 1) * 128, 32:64])

        zt = zp.tile([128, 132, 132], F16)
        tt = tp.tile([128, 131, 128], F16)

        def emit_block(ct, k):
            """Matmuls + psum->z copies for fine rows 12k..12k+11."""
            for a in range(2):
                for b in range(2):
                    ps = pp.tile([128, 6, 66], F32)
                    taps = [(ky, ry, kx, rx)
                            for (ky, ry) in ROWTAPS[a]
                            for (kx, rx) in ROWTAPS[b]]
                    n = len(taps) * 4
                    i = 0
                    for cg in range(4):
                        for (ky, ry, kx, rx) in taps:
                            nc.tensor.matmul(
                                ps[:],
                                wsb[:, ct, cg, ky * 3 + kx, :],
                                xq[:, cg, 6 * k + ry:6 * k + ry + 6,
                                   rx:rx + 66],
                                start=(i == 0), stop=(i == n - 1))
                            i += 1
                    nc.scalar.copy(
                        zt[:, 12 * k + a:12 * k + 12:2, b:132:2], ps[:])

        def emit_row_chunk(i):
            """Horizontal FIR t[r,c] = (z[c]+3z[c+1]) + (3z[c+2]+z[c+3])."""
            r0, nr = 16 * i, (16 if i < 7 else 19)
            z3 = slp.tile([128, 19, 132], F16, tag="sl")
            nc.vector.tensor_scalar_mul(z3[:, 0:nr, :], zt[:, r0:r0 + nr, :],
                                        3.0)
            u = adp.tile([128, 19, 128], F16, tag="ad")
            nc.vector.tensor_tensor(u[:, 0:nr, :], zt[:, r0:r0 + nr, 0:128],
                                    z3[:, 0:nr, 1:129], ADD)
            v = adp.tile([128, 19, 128], F16, tag="ad")
            nc.vector.tensor_tensor(v[:, 0:nr, :], z3[:, 0:nr, 2:130],
                                    zt[:, r0:r0 + nr, 3:131], ADD)
            nc.vector.tensor_tensor(tt[:, r0:r0 + nr, :], u[:, 0:nr, :],
                                    v[:, 0:nr, :], ADD)

        def emit_col_fir(ct, i):
            """Vertical FIR + noise add for out rows 16i..+16 -> nadd tile."""
            r0 = 16 * i
            t3 = slp.tile([128, 19, 128], F16, tag="sl")
            nc.vector.tensor_scalar_mul(t3[:], tt[:, r0:r0 + 19, :], 3.0)
            u2 = adp.tile([128, 19, 128], F16, tag="ad")
            nc.vector.tensor_tensor(u2[:, 0:16, :], tt[:, r0:r0 + 16, :],
                                    t3[:, 1:17, :], ADD)
            v2 = adp.tile([128, 19, 128], F16, tag="ad")
            nc.vector.tensor_tensor(v2[:, 0:16, :], t3[:, 2:18, :],
                                    tt[:, r0 + 3:r0 + 19, :], ADD)
            yr = yrp.tile([128, 16, 128], F16)
            nc.vector.tensor_tensor(yr[:], u2[:, 0:16, :], v2[:, 0:16, :], ADD)
            nzt = nzp.tile([128, 16, 128], F16)
            nsrc = bass.AP(tensor=noise.tensor, offset=r0 * 128,
                           ap=[[0, 128], [128, 16], [1, 128]])
            nc.sync.dma_start(nzt[:], nsrc)
            na = nap.tile([128, 16, 128], F16)
            nc.gpsimd.tensor_tensor(na[:], yr[:], nzt[:], ADD)
            return na

        def emit_epilogue(ct, i, na):
            """Prelu (scale+bias+lrelu+fp32) and output DMA for chunk i."""
            r0 = 16 * i
            ys = ysp.tile([128, 16, 128], F32)
            nc.scalar.activation(ys[:], na[:],
                                 mybir.ActivationFunctionType.Prelu,
                                 bias=bias_sb[:, ct:ct + 1], scale=ACT_SCALE,
                                 alpha=0.2)
            ydst = bass.AP(tensor=y.tensor,
                           offset=(ct * 128) * 16384 + r0 * 128,
                           ap=[[16384, 128], [128, 16], [1, 128]])
            nc.sync.dma_start(ydst, ys[:])

        # flat schedule over 22 blocks; epilogues ride EPI_DELAY slots late
        pending: dict[int, list] = {}
        for f, (ct, k) in enumerate([(c, k) for c in range(2)
                                     for k in range(11)]):
            emit_block(ct, k)
            for i in ROW_SCHED.get(k, []):
                emit_row_chunk(i)
            for i in COL_SCHED.get(k, []):
                na = emit_col_fir(ct, i)
                pending.setdefault(f + EPI_DELAY, []).append((ct, i, na))
            for (pct, pi, pna) in pending.pop(f, []):
                emit_epilogue(pct, pi, pna)
        for fl in sorted(pending):
            for (pct, pi, pna) in pending.pop(fl):
                emit_epilogue(pct, pi, pna)
    legalize_waits(nc)
    return nc


# ---------------------------------------------------------------------------
# Host-side preparation (weight renorm + polyphase layout, fp16 conversion)
# ---------------------------------------------------------------------------

def prep_inputs(x, weight, bias, noise_const, noise_strength):
    SQ2 = np.sqrt(2.0)
    w = np.asarray(weight).astype(np.float64)
    inv = 1.0 / np.sqrt((w ** 2).sum(axis=(1, 2, 3)) + 1e-8)
    w = w * inv[:, None, None, None]
    wf = w[:, :, ::-1, ::-1]                    # flipped (cross-corr of flip)

    # wq[ci_local, ct, cg, ky*3+kx, co] = wf[ct*128+co, cg*128+ci, ky, kx]
    wq = np.empty((128, 2, 4, 9, 128), dtype=np.float16)
    for ct in range(2):
        for cg in range(4):
            blk = wf[ct * 128:(ct + 1) * 128, cg * 128:(cg + 1) * 128]
            wq[:, ct, cg, :, :] = blk.transpose(1, 2, 3, 0).reshape(128, 9, 128)

    # noise pre-scaled so the Act scale sqrt2/16 lands it at
    # sqrt2 * strength * noise:  noise_pre = 16 * strength * noise
    noise_pre = (np.asarray(noise_const).astype(np.float64)
                 * float(noise_strength) * 16.0).astype(np.float16)

    biasq = np.empty((128, 2), dtype=np.float32)
    bias2 = np.asarray(bias).astype(np.float64) * SQ2
    biasq[:, 0] = bias2[:128]
    biasq[:, 1] = bias2[128:]

    x = np.asarray(x)
    return [{
        "xin": x[bi].astype(np.float16),
        "wq": wq,
        "noise": noise_pre,
        "biasq": biasq,
    } for bi in range(x.shape[0])]


_NC_CACHE = None


def kernel(x, weight, bias, noise_const, noise_strength):
    global _NC_CACHE
    if _NC_CACHE is None:
        _NC_CACHE = build_conv_nc()
    in_maps = prep_inputs(x, weight, bias, noise_const, noise_strength)
    res = run_bass_kernel_spmd(_NC_CACHE, in_maps, core_ids=list(range(8)))
    return np.ascontiguousarray(
        np.stack([r["y"] for r in res.results]), dtype=np.float32)
